# revision 26
# baseline (speedup 1.0000x reference)
"""Trainium2 Bass kernel for nn_ModeSelection (mode-selection MLP + one-hot gather).

Math (per batch row b, paths p=0..9):
  ctx[b,p,:]  = concat(agent[b], lane[b,p], ngh_lane[b,p], ngh[b,p])   # [2048]
  emb[b,p,:]  = relu(ctx[b,p,:] @ W1 + b1)                             # [512]
  logits[b,:] = emb[b].reshape(5120) @ W2 + b2                         # [10]
  idx[b]      = argmax(lane_label[b])  (first max)
  best_*[b]   = {lane, ngh_lane, ngh}[b, idx[b]]

Sharding: pure data parallel over batch (4096 -> 8 cores x 512 rows).
Weights replicated. No collectives.

Device strategy per core:
  - ctx parts are DMA'd straight from HBM in TRANSPOSED layout (bf16 XBAR
    DMA transpose) -> rhs operands [feat, rowpath] for the tensor engine.
  - mm1 accumulates over 16 K-blocks of 128 into PSUM [128, 320] tiles,
    relu+b1 applied by the scalar engine on the way to SBUF (bf16 embT).
  - mm2 contracts embT [feat, rowpath] with W2 into logitsT [10, 512],
    +b2 and a tiny PE transpose produce natural-layout logits.
  - best_* gathers: argmax computed on DVE from the labels, rows fetched
    by GPSIMD indirect DMA from the fp32 copies in HBM (bit-exact).
"""

import sys

sys.path.insert(0, "/opt/trn_rl_repo")

import numpy as np
import ml_dtypes

import concourse.bass as bass
import concourse.mybir as mybir
from concourse import bacc, tile
from concourse.bass_utils import run_bass_kernel_spmd
from concourse.masks import make_identity
from concourse.tile_rust import add_dep_helper

BF16 = ml_dtypes.bfloat16

N_CORES = 8
B = 4096          # full batch
BSH = B // N_CORES  # 512 batch rows per core
P_PATHS = 10
LD = 512          # feature dim of each ctx part
DIN = 2048
RP = BSH * P_PATHS  # 5120 rowpath (b,p) rows per core

NCHUNK = 8
CB = BSH // NCHUNK   # 64 batch rows per chunk
RC = CB * P_PATHS    # 640 rowpath rows per chunk
NG = 2               # matmul groups per chunk
GN = RC // NG        # 320 rowpath per matmul group

F32 = mybir.dt.float32
BF = mybir.dt.bfloat16
I32 = mybir.dt.int32

_CACHE = {}


def _build():
    nc = bacc.Bacc("TRN2", target_bir_lowering=False, debug=False,
                   enable_asserts=False, num_devices=N_CORES)

    # inputs
    agent_n = nc.dram_tensor("agent_n", [128, 4, LD], BF, kind="ExternalInput").ap()
    parts_bf = [
        nc.dram_tensor(name, [RP, LD], BF, kind="ExternalInput").ap()
        for name in ("lane_bf", "nghl_bf", "ngh_bf")
    ]
    gather_f = [
        nc.dram_tensor(name, [RP, LD], F32, kind="ExternalInput").ap()
        for name in ("lane_f", "nghl_f", "ngh_f")
    ]
    label = nc.dram_tensor("label", [BSH, P_PATHS], I32, kind="ExternalInput").ap()
    w1 = nc.dram_tensor("w1", [128, 16, 512], BF, kind="ExternalInput").ap()
    w2 = nc.dram_tensor("w2", [128, 40, 10], BF, kind="ExternalInput").ap()
    b1 = nc.dram_tensor("b1", [128, 4], F32, kind="ExternalInput").ap()
    b2 = nc.dram_tensor("b2", [10, 1], F32, kind="ExternalInput").ap()

    # outputs
    logits_d = nc.dram_tensor("logits", [BSH, P_PATHS], F32, kind="ExternalOutput").ap()
    best_d = [
        nc.dram_tensor(name, [BSH, LD], F32, kind="ExternalOutput").ap()
        for name in ("best_lane", "best_nghl", "best_ngh")
    ]

    with tile.TileContext(nc) as tc:
        with (
            tc.tile_pool(name="const", bufs=1) as const,
            tc.tile_pool(name="ctxt", bufs=3) as ctxt_pool,
            tc.tile_pool(name="mmps", bufs=6, space="PSUM") as mm_psum,
            tc.tile_pool(name="ltps", bufs=1, space="PSUM") as lt_psum,
            tc.tile_pool(name="tpps", bufs=1, space="PSUM") as tp_psum,
            tc.tile_pool(name="work", bufs=2) as work,
            tc.tile_pool(name="gat", bufs=6) as gat,
        ):
            # ---- startup. Plain loads only (W1 first: it gates all matmuls).
            # The agent matrix is transposed on the PE (not the XBAR) so the
            # chunk-0 DMA-transposes only have to wait for the plain loads to
            # drain once; everything else (W2, b2, labels, gather) is deferred
            # behind the last XBAR transpose — plain DMA traffic concurrent
            # with DMA-transposes serializes the XBAR pipeline.
            w1t = const.tile([128, 16, 512], BF)
            nc.scalar.dma_start(w1t[:], w1[:])
            ag_nat = const.tile([128, 4, LD], BF)
            nc.gpsimd.dma_start(ag_nat[:], agent_n[:])
            b1t = const.tile([128, 4], F32)
            nc.scalar.dma_start(b1t[:], b1[:])
            id10 = const.tile([10, 10], F32)
            make_identity(nc, id10[:])
            id128 = const.tile([128, 128], BF)
            make_identity(nc, id128[:])
            wvec = const.tile([128, 10], I32)  # [10, 9, ..., 1] per partition
            nc.gpsimd.iota(wvec[:], pattern=[[-1, 10]], base=10, channel_multiplier=0)
            embT = const.tile([128, 4, RP], BF)  # persistent emb^T, 40KB/partition

            # ---- gather indices: labels land with the startup plain loads,
            # argmax chain runs on the then-idle DVE. Only the indirect
            # gathers + stores are deferred behind the XBAR transposes.
            rowidxs = []
            for blk in range(4):
                lblt = gat.tile([128, 10], I32, tag="lbl")
                nc.sync.dma_start(lblt[:], label[blk * 128:(blk + 1) * 128, :])
                score = gat.tile([128, 10], I32, tag="score")
                nc.vector.tensor_tensor(out=score[:], in0=lblt[:], in1=wvec[:],
                                        op=mybir.AluOpType.mult)
                m = gat.tile([128, 1], I32, tag="m")
                nc.vector.tensor_reduce(out=m[:], in_=score[:],
                                        axis=mybir.AxisListType.X,
                                        op=mybir.AluOpType.max)
                # idx = (10 - m) * min(m, 1): all-zero label row -> m=0 -> idx 0,
                # first set bit otherwise — matches argmax tie-breaking.
                u = gat.tile([128, 1], I32, tag="u")
                nc.vector.tensor_scalar(out=u[:], in0=m[:], scalar1=-1, scalar2=10,
                                        op0=mybir.AluOpType.mult,
                                        op1=mybir.AluOpType.add)
                gz = gat.tile([128, 1], I32, tag="gz")
                nc.vector.tensor_scalar(out=gz[:], in0=m[:], scalar1=1, scalar2=None,
                                        op0=mybir.AluOpType.min)
                idxv = gat.tile([128, 1], I32, tag="idx")
                nc.vector.tensor_tensor(out=idxv[:], in0=u[:], in1=gz[:],
                                        op=mybir.AluOpType.mult)
                base = gat.tile([128, 1], I32, tag="base")
                nc.gpsimd.iota(base[:], pattern=[[0, 1]], base=blk * 1280,
                               channel_multiplier=10)
                rowidx = gat.tile([128, 1], I32, tag="rowidx")
                nc.vector.tensor_tensor(out=rowidx[:], in0=idxv[:], in1=base[:],
                                        op=mybir.AluOpType.add)
                rowidxs.append(rowidx)

            # ---- agent transpose on PE + pre-matmul: agA[of, b] = agent @ W1[:512]
            agT = const.tile([128, 4, BSH], BF)
            for kb in range(4):
                tps = mm_psum.tile([128, BSH], BF, tag="ps")
                for s in range(4):
                    nc.tensor.transpose(
                        tps[:, s * 128:(s + 1) * 128],
                        ag_nat[:, s, kb * 128:(kb + 1) * 128], id128[:])
                if kb % 2 == 0:
                    nc.vector.tensor_copy(agT[:, kb, :], tps[:])
                else:
                    nc.scalar.copy(agT[:, kb, :], tps[:])
            agA = const.tile([128, 4, BSH], BF)
            for fbo in range(4):
                agps = mm_psum.tile([128, BSH], F32, tag="ps")
                for kb in range(4):
                    nc.tensor.matmul(
                        agps[:],
                        lhsT=w1t[:, kb, fbo * 128:(fbo + 1) * 128],
                        rhs=agT[:, kb, :],
                        start=(kb == 0), stop=(kb == 3),
                    )
                if fbo % 2 == 0:
                    nc.vector.tensor_copy(agA[:, fbo, :], agps[:])
                else:
                    nc.scalar.copy(agA[:, fbo, :], agps[:])

            # ---- mm1: embT[of, (b,p)] = relu(sum_part ctxT @ W1 + agA + b1) ----
            last_tp = None
            for c in range(NCHUNK):
                ctxTs = []
                for pi, part in enumerate(parts_bf):
                    t = ctxt_pool.tile([128, 4, RC], BF, tag=f"ctxt{pi}")
                    last_tp = nc.sync.dma_start_transpose(
                        t[:], part[c * RC:(c + 1) * RC, :])
                    ctxTs.append(t)
                # replicate this chunk's agent rows 10x (contiguous rhs for the
                # accumulating agent matmul below); step-0 reads are cheap on
                # DVE, fatal to matmul streaming rate
                agrep = ctxt_pool.tile([128, 4, RC], BF, tag="agrep")
                for fb in range(4):
                    ag_src = (agA[:, fb, c * CB:(c + 1) * CB]
                              .rearrange("p (b o) -> p b o", o=1)
                              .to_broadcast([128, CB, P_PATHS]))
                    nc.vector.tensor_copy(
                        agrep[:, fb, :].rearrange("p (b o) -> p b o", o=P_PATHS),
                        ag_src)
                for fbo in range(4):
                    for g in range(NG):
                        ps = mm_psum.tile([128, GN], F32)
                        k = 0
                        for pi, t in enumerate(ctxTs):
                            for kb in range(4):
                                nc.tensor.matmul(
                                    ps[:],
                                    lhsT=w1t[:, 4 + pi * 4 + kb,
                                             fbo * 128:(fbo + 1) * 128],
                                    rhs=t[:, kb, g * GN:(g + 1) * GN],
                                    start=(k == 0), stop=False,
                                )
                                k += 1
                        # agent contribution via identity-stationary matmul
                        nc.tensor.matmul(
                            ps[:], lhsT=id128[:],
                            rhs=agrep[:, fbo, g * GN:(g + 1) * GN],
                            start=False, stop=True)
                        off = c * RC + g * GN
                        nc.scalar.activation(
                            embT[:, fbo, off:off + GN], ps[:],
                            mybir.ActivationFunctionType.Relu,
                            bias=b1t[:, fbo:fbo + 1],
                        )

            def after_tp(instr, why):
                add_dep_helper(instr.ins, last_tp.ins, sync=True, reason=why)
                return instr

            # ---- deferred plain loads (post-transpose window) ----
            w2t = const.tile([128, 40, 10], BF)
            after_tp(nc.scalar.dma_start(w2t[:], w2[:]), "defer w2 after xbar")
            b2t = const.tile([10, 1], F32)
            after_tp(nc.scalar.dma_start(b2t[:], b2[:]), "defer b2 after xbar")

            # ---- deferred indirect gathers + stores (post-transpose) ----
            for blk in range(4):
                for ti, (src_t, dst) in enumerate(zip(gather_f, best_d)):
                    gt = gat.tile([128, LD], F32, tag="g")
                    after_tp(nc.gpsimd.indirect_dma_start(
                        out=gt[:], out_offset=None, in_=src_t[:],
                        in_offset=bass.IndirectOffsetOnAxis(
                            ap=rowidxs[blk][:, :1], axis=0)),
                        "defer gather after xbar")
                    nc.gpsimd.dma_start(dst[blk * 128:(blk + 1) * 128, :], gt[:])

            # ---- mm2: logitsT[10, b] = sum_{fb, pp} W2_blk^T @ embT_strided ----
            lt_ps = lt_psum.tile([10, BSH], F32, tag="ltmix")
            k = 0
            for fb in range(4):
                embT_fb = embT[:, fb, :].rearrange("p (b t) -> p b t", t=P_PATHS)
                for pp in range(P_PATHS):
                    nc.tensor.matmul(
                        lt_ps[:],
                        lhsT=w2t[:, pp * 4 + fb, :],
                        rhs=embT_fb[:, :, pp],
                        start=(k == 0), stop=(k == 39),
                    )
                    k += 1
            lt_sb = work.tile([10, BSH], F32, tag="ltsb")
            nc.scalar.activation(lt_sb[:], lt_ps[:],
                                 mybir.ActivationFunctionType.Identity,
                                 bias=b2t[:, 0:1])
            # transpose to natural [b, 10] layout and store
            lg = work.tile([128, 4, 10], F32, tag="lg")
            for j in range(4):
                tp = tp_psum.tile([128, 10], F32)
                nc.tensor.transpose(tp[:], lt_sb[:, j * 128:(j + 1) * 128], id10[:])
                nc.vector.tensor_copy(lg[:, j, :], tp[:])
            nc.scalar.dma_start(logits_d.rearrange("(j p) n -> p j n", p=128), lg[:])

    nc.compile()
    return nc


def _get_nc():
    if "nc" not in _CACHE:
        _CACHE["nc"] = _build()
    return _CACHE["nc"]


def _prepare_in_maps(agent_context, lane_contexts, ngh_lane_context, ngh_contexts,
                     lane_label, W1, b1, W2, b2):
    agent_context = np.asarray(agent_context, np.float32)
    lane_contexts = np.asarray(lane_contexts, np.float32)
    ngh_lane_context = np.asarray(ngh_lane_context, np.float32)
    ngh_contexts = np.asarray(ngh_contexts, np.float32)
    lane_label = np.asarray(lane_label, np.int32)
    W1 = np.asarray(W1, np.float32)
    b1 = np.asarray(b1, np.float32)
    W2 = np.asarray(W2, np.float32)
    b2 = np.asarray(b2, np.float32)

    # replicated weights, pre-arranged for SBUF block layout
    w1_r = np.ascontiguousarray(
        W1.reshape(16, 128, 512).transpose(1, 0, 2)).astype(BF16)
    w2_r = np.ascontiguousarray(
        W2.reshape(40, 128, 10).transpose(1, 0, 2)).astype(BF16)
    b1_r = np.ascontiguousarray(b1.reshape(4, 128).T)
    b2_r = np.ascontiguousarray(b2.reshape(10, 1))

    in_maps = []
    for c in range(N_CORES):
        sl = slice(c * BSH, (c + 1) * BSH)
        lane = np.ascontiguousarray(lane_contexts[sl].reshape(RP, LD))
        nghl = np.ascontiguousarray(ngh_lane_context[sl].reshape(RP, LD))
        ngh = np.ascontiguousarray(ngh_contexts[sl].reshape(RP, LD))
        in_maps.append({
            "agent_n": np.ascontiguousarray(
                agent_context[sl].reshape(4, 128, LD).transpose(1, 0, 2)).astype(BF16),
            "lane_bf": lane.astype(BF16),
            "nghl_bf": nghl.astype(BF16),
            "ngh_bf": ngh.astype(BF16),
            "lane_f": lane,
            "nghl_f": nghl,
            "ngh_f": ngh,
            "label": np.ascontiguousarray(lane_label[sl]),
            "w1": w1_r, "w2": w2_r, "b1": b1_r, "b2": b2_r,
        })
    return in_maps


def _assemble(outs):

    logits = np.concatenate([outs[c]["logits"] for c in range(N_CORES)], axis=0)
    best_lane = np.concatenate([outs[c]["best_lane"] for c in range(N_CORES)], axis=0)
    best_nghl = np.concatenate([outs[c]["best_nghl"] for c in range(N_CORES)], axis=0)
    best_ngh = np.concatenate([outs[c]["best_ngh"] for c in range(N_CORES)], axis=0)
    return logits, best_lane, best_nghl, best_ngh


def _make_runner(nc):
    """Build a persistently-cached jitted SPMD runner (same mechanism as
    bass2jax.run_bass_via_pjrt, but the jit closure survives across calls so
    repeat invocations skip retracing)."""
    import jax
    from jax.sharding import Mesh, PartitionSpec
    from jax.experimental.shard_map import shard_map
    from concourse import bass2jax, mybir as mb

    bass2jax.install_neuronx_cc_hook()
    partition_name = nc.partition_id_tensor.name if nc.partition_id_tensor else None

    in_names, out_names, out_avals, zero_shapes = [], [], [], []
    for alloc in nc.m.functions[0].allocations:
        if not isinstance(alloc, mb.MemoryLocationSet):
            continue
        name = alloc.memorylocations[0].name
        if alloc.kind == "ExternalInput":
            if name != partition_name:
                in_names.append(name)
        elif alloc.kind == "ExternalOutput":
            shape = tuple(alloc.tensor_shape)
            dtype = mb.dt.np(alloc.dtype)
            out_names.append(name)
            out_avals.append(jax.core.ShapedArray(shape, dtype))
            zero_shapes.append((shape, dtype))
    n_params = len(in_names)
    n_outs = len(out_names)
    all_in_names = list(in_names) + list(out_names)
    if partition_name is not None:
        all_in_names.append(partition_name)

    def _body(*args):
        operands = list(args)
        if partition_name is not None:
            operands.append(bass2jax.partition_id_tensor())
        outs = bass2jax._bass_exec_p.bind(
            *operands,
            out_avals=tuple(out_avals),
            in_names=tuple(all_in_names),
            out_names=tuple(out_names),
            lowering_input_output_aliases=(),
            sim_require_finite=True,
            sim_require_nnan=True,
            nc=nc,
        )
        return tuple(outs)

    devices = jax.devices()[:N_CORES]
    mesh = Mesh(np.asarray(devices), ("core",))
    in_specs = (PartitionSpec("core"),) * (n_params + n_outs)
    out_specs = (PartitionSpec("core"),) * n_outs
    donate = tuple(range(n_params, n_params + n_outs))
    sharded = jax.jit(
        shard_map(_body, mesh=mesh, in_specs=in_specs, out_specs=out_specs,
                  check_rep=False),
        donate_argnums=donate, keep_unused=True)

    def run(in_maps):
        concat_in = [
            np.concatenate([np.asarray(in_maps[c][n]) for c in range(N_CORES)], axis=0)
            for n in in_names
        ]
        concat_zeros = [
            np.zeros((N_CORES * s[0], *s[1:]), d) for (s, d) in zero_shapes
        ]
        out_arrs = sharded(*concat_in, *concat_zeros)
        return [
            {n: np.asarray(out_arrs[i]).reshape(N_CORES, *zero_shapes[i][0])[c]
             for i, n in enumerate(out_names)}
            for c in range(N_CORES)
        ]

    return run


def _get_runner():
    if "runner" not in _CACHE:
        nc = _get_nc()
        try:
            _CACHE["runner"] = _make_runner(nc)
        except Exception:
            _CACHE["runner"] = None
    return _CACHE["runner"]


def kernel(**inputs):
    nc = _get_nc()
    in_maps = _prepare_in_maps(**inputs)
    runner = _get_runner()
    if runner is not None:
        try:
            return _assemble(runner(in_maps))
        except Exception:
            _CACHE["runner"] = None
    res = run_bass_kernel_spmd(nc, in_maps, core_ids=list(range(N_CORES)))
    return _assemble(res.results)


# revision 27
# speedup vs baseline: 1.1376x; 1.1376x over previous
"""Trainium2 Bass kernel for nn_ModeSelection (mode-selection MLP + one-hot gather).

Math (per batch row b, paths p=0..9):
  ctx[b,p,:]  = concat(agent[b], lane[b,p], ngh_lane[b,p], ngh[b,p])   # [2048]
  emb[b,p,:]  = relu(ctx[b,p,:] @ W1 + b1)                             # [512]
  logits[b,:] = emb[b].reshape(5120) @ W2 + b2                         # [10]
  idx[b]      = argmax(lane_label[b])  (first max)
  best_*[b]   = {lane, ngh_lane, ngh}[b, idx[b]]

Sharding: pure data parallel over batch (4096 -> 8 cores x 512 rows).
Weights replicated. No collectives.

Device strategy per core:
  - ctx parts are DMA'd straight from HBM in TRANSPOSED layout (bf16 XBAR
    DMA transpose) -> rhs operands [feat, rowpath] for the tensor engine.
  - mm1 accumulates over 16 K-blocks of 128 into PSUM [128, 320] tiles,
    relu+b1 applied by the scalar engine on the way to SBUF (bf16 embT).
  - mm2 contracts embT [feat, rowpath] with W2 into logitsT [10, 512],
    +b2 and a tiny PE transpose produce natural-layout logits.
  - best_* gathers: argmax computed on DVE from the labels, rows fetched
    by GPSIMD indirect DMA from the fp32 copies in HBM (bit-exact).
"""

import sys

sys.path.insert(0, "/opt/trn_rl_repo")

import numpy as np
import ml_dtypes

import concourse.bass as bass
import concourse.mybir as mybir
from concourse import bacc, tile
from concourse.bass_utils import run_bass_kernel_spmd
from concourse.masks import make_identity
from concourse.tile_rust import add_dep_helper

BF16 = ml_dtypes.bfloat16

N_CORES = 8
B = 4096          # full batch
BSH = B // N_CORES  # 512 batch rows per core
P_PATHS = 10
LD = 512          # feature dim of each ctx part
DIN = 2048
RP = BSH * P_PATHS  # 5120 rowpath (b,p) rows per core

NCHUNK = 8
CB = BSH // NCHUNK   # 64 batch rows per chunk
RC = CB * P_PATHS    # 640 rowpath rows per chunk
NG = 2               # matmul groups per chunk
GN = RC // NG        # 320 rowpath per matmul group

F32 = mybir.dt.float32
BF = mybir.dt.bfloat16
I32 = mybir.dt.int32

_CACHE = {}


def _build():
    nc = bacc.Bacc("TRN2", target_bir_lowering=False, debug=False,
                   enable_asserts=False, num_devices=N_CORES)

    # inputs
    agent_n = nc.dram_tensor("agent_n", [128, 4, LD], BF, kind="ExternalInput").ap()
    parts_bf = [
        nc.dram_tensor(name, [RP, LD], BF, kind="ExternalInput").ap()
        for name in ("lane_bf", "nghl_bf", "ngh_bf")
    ]
    gather_f = [
        nc.dram_tensor(name, [RP, LD], F32, kind="ExternalInput").ap()
        for name in ("lane_f", "nghl_f", "ngh_f")
    ]
    label = nc.dram_tensor("label", [BSH, P_PATHS], I32, kind="ExternalInput").ap()
    w1 = nc.dram_tensor("w1", [128, 16, 512], BF, kind="ExternalInput").ap()
    w2 = nc.dram_tensor("w2", [128, 40, 10], BF, kind="ExternalInput").ap()
    b1 = nc.dram_tensor("b1", [128, 4], F32, kind="ExternalInput").ap()
    b2 = nc.dram_tensor("b2", [10, 1], F32, kind="ExternalInput").ap()

    # outputs
    logits_d = nc.dram_tensor("logits", [BSH, P_PATHS], F32, kind="ExternalOutput").ap()
    best_d = [
        nc.dram_tensor(name, [BSH, LD], F32, kind="ExternalOutput").ap()
        for name in ("best_lane", "best_nghl", "best_ngh")
    ]

    with tile.TileContext(nc) as tc:
        with (
            tc.tile_pool(name="const", bufs=1) as const,
            tc.tile_pool(name="ctxt", bufs=3) as ctxt_pool,
            tc.tile_pool(name="mmps", bufs=6, space="PSUM") as mm_psum,
            tc.tile_pool(name="ltps", bufs=1, space="PSUM") as lt_psum,
            tc.tile_pool(name="tpps", bufs=1, space="PSUM") as tp_psum,
            tc.tile_pool(name="work", bufs=2) as work,
            tc.tile_pool(name="gat", bufs=6) as gat,
        ):
            # ---- startup. Plain loads only (W1 first: it gates all matmuls).
            # The agent matrix is transposed on the PE (not the XBAR) so the
            # chunk-0 DMA-transposes only have to wait for the plain loads to
            # drain once; everything else (W2, b2, labels, gather) is deferred
            # behind the last XBAR transpose — plain DMA traffic concurrent
            # with DMA-transposes serializes the XBAR pipeline.
            w1t = const.tile([128, 16, 512], BF)
            nc.scalar.dma_start(w1t[:], w1[:])
            ag_nat = const.tile([128, 4, LD], BF)
            nc.gpsimd.dma_start(ag_nat[:], agent_n[:])
            b1t = const.tile([128, 4], F32)
            nc.scalar.dma_start(b1t[:], b1[:])
            id10 = const.tile([10, 10], F32)
            make_identity(nc, id10[:])
            id128 = const.tile([128, 128], BF)
            make_identity(nc, id128[:])
            wvec = const.tile([128, 10], I32)  # [10, 9, ..., 1] per partition
            nc.gpsimd.iota(wvec[:], pattern=[[-1, 10]], base=10, channel_multiplier=0)
            embT = const.tile([128, 4, P_PATHS, BSH], BF)  # emb^T, path-major

            # ---- gather indices: labels land with the startup plain loads,
            # argmax chain runs on the then-idle DVE. Only the indirect
            # gathers + stores are deferred behind the XBAR transposes.
            rowidxs = []
            for blk in range(4):
                lblt = gat.tile([128, 10], I32, tag="lbl")
                nc.sync.dma_start(lblt[:], label[blk * 128:(blk + 1) * 128, :])
                score = gat.tile([128, 10], I32, tag="score")
                nc.vector.tensor_tensor(out=score[:], in0=lblt[:], in1=wvec[:],
                                        op=mybir.AluOpType.mult)
                m = gat.tile([128, 1], I32, tag="m")
                nc.vector.tensor_reduce(out=m[:], in_=score[:],
                                        axis=mybir.AxisListType.X,
                                        op=mybir.AluOpType.max)
                # idx = (10 - m) * min(m, 1): all-zero label row -> m=0 -> idx 0,
                # first set bit otherwise — matches argmax tie-breaking.
                u = gat.tile([128, 1], I32, tag="u")
                nc.vector.tensor_scalar(out=u[:], in0=m[:], scalar1=-1, scalar2=10,
                                        op0=mybir.AluOpType.mult,
                                        op1=mybir.AluOpType.add)
                gz = gat.tile([128, 1], I32, tag="gz")
                nc.vector.tensor_scalar(out=gz[:], in0=m[:], scalar1=1, scalar2=None,
                                        op0=mybir.AluOpType.min)
                idxv = gat.tile([128, 1], I32, tag="idx")
                nc.vector.tensor_tensor(out=idxv[:], in0=u[:], in1=gz[:],
                                        op=mybir.AluOpType.mult)
                base = gat.tile([128, 1], I32, tag="base")
                nc.gpsimd.iota(base[:], pattern=[[0, 1]], base=blk * 1280,
                               channel_multiplier=10)
                rowidx = gat.tile([128, 1], I32, tag="rowidx")
                nc.vector.tensor_tensor(out=rowidx[:], in0=idxv[:], in1=base[:],
                                        op=mybir.AluOpType.add)
                rowidxs.append(rowidx)

            # ---- agent transpose on PE + pre-matmul: agA[of, b] = agent @ W1[:512]
            agT = const.tile([128, 4, BSH], BF)
            for kb in range(4):
                tps = mm_psum.tile([128, BSH], BF, tag="ps")
                for s in range(4):
                    nc.tensor.transpose(
                        tps[:, s * 128:(s + 1) * 128],
                        ag_nat[:, s, kb * 128:(kb + 1) * 128], id128[:])
                if kb % 2 == 0:
                    nc.vector.tensor_copy(agT[:, kb, :], tps[:])
                else:
                    nc.scalar.copy(agT[:, kb, :], tps[:])
            agA = const.tile([128, 4, BSH], BF)
            for fbo in range(4):
                agps = mm_psum.tile([128, BSH], F32, tag="ps")
                for kb in range(4):
                    nc.tensor.matmul(
                        agps[:],
                        lhsT=w1t[:, kb, fbo * 128:(fbo + 1) * 128],
                        rhs=agT[:, kb, :],
                        start=(kb == 0), stop=(kb == 3),
                    )
                if fbo % 2 == 0:
                    nc.vector.tensor_copy(agA[:, fbo, :], agps[:])
                else:
                    nc.scalar.copy(agA[:, fbo, :], agps[:])

            # ---- mm1: embT[of, (b,p)] = relu(sum_part ctxT @ W1 + agA + b1) ----
            last_tp = None
            for c in range(NCHUNK):
                ctxTs = []
                for pi, part in enumerate(parts_bf):
                    t = ctxt_pool.tile([128, 4, RC], BF, tag=f"ctxt{pi}")
                    last_tp = nc.sync.dma_start_transpose(
                        t[:], part[c * RC:(c + 1) * RC, :])
                    ctxTs.append(t)
                # replicate this chunk's agent rows 10x (contiguous rhs for the
                # accumulating agent matmul below); step-0 reads are cheap on
                # DVE, fatal to matmul streaming rate
                agrep = ctxt_pool.tile([128, 4, RC], BF, tag="agrep")
                for fb in range(4):
                    ag_src = (agA[:, fb, c * CB:(c + 1) * CB]
                              .rearrange("p (b o) -> p b o", o=1)
                              .to_broadcast([128, CB, P_PATHS]))
                    nc.vector.tensor_copy(
                        agrep[:, fb, :].rearrange("p (b o) -> p b o", o=P_PATHS),
                        ag_src)
                for fbo in range(4):
                    for g in range(NG):
                        ps = mm_psum.tile([128, GN], F32)
                        k = 0
                        for pi, t in enumerate(ctxTs):
                            for kb in range(4):
                                nc.tensor.matmul(
                                    ps[:],
                                    lhsT=w1t[:, 4 + pi * 4 + kb,
                                             fbo * 128:(fbo + 1) * 128],
                                    rhs=t[:, kb, g * GN:(g + 1) * GN],
                                    start=(k == 0), stop=False,
                                )
                                k += 1
                        b0 = c * CB + g * (GN // P_PATHS)
                        # agent contribution via identity-stationary matmul
                        nc.tensor.matmul(
                            ps[:], lhsT=id128[:],
                            rhs=agrep[:, fbo, g * GN:(g + 1) * GN],
                            start=False, stop=True)
                        nc.scalar.activation(
                            embT[:, fbo, :, b0:b0 + GN // P_PATHS]
                            .rearrange("p t b -> p b t"), ps[:],
                            mybir.ActivationFunctionType.Relu,
                            bias=b1t[:, fbo:fbo + 1],
                        )

            def after_tp(instr, why):
                add_dep_helper(instr.ins, last_tp.ins, sync=True, reason=why)
                return instr

            # ---- deferred plain loads (post-transpose window) ----
            w2t = const.tile([128, 40, 10], BF)
            after_tp(nc.scalar.dma_start(w2t[:], w2[:]), "defer w2 after xbar")
            b2t = const.tile([10, 1], F32)
            after_tp(nc.scalar.dma_start(b2t[:], b2[:]), "defer b2 after xbar")

            # ---- deferred indirect gathers + stores (post-transpose) ----
            for blk in range(4):
                for ti, (src_t, dst) in enumerate(zip(gather_f, best_d)):
                    gt = gat.tile([128, LD], F32, tag="g")
                    after_tp(nc.gpsimd.indirect_dma_start(
                        out=gt[:], out_offset=None, in_=src_t[:],
                        in_offset=bass.IndirectOffsetOnAxis(
                            ap=rowidxs[blk][:, :1], axis=0)),
                        "defer gather after xbar")
                    nc.gpsimd.dma_start(dst[blk * 128:(blk + 1) * 128, :], gt[:])

            # ---- mm2: logitsT[10, b] = sum_{fb, pp} W2_blk^T @ embT_strided ----
            lt_ps = lt_psum.tile([10, BSH], F32, tag="ltmix")
            k = 0
            for fb in range(4):
                for pp in range(P_PATHS):
                    nc.tensor.matmul(
                        lt_ps[:],
                        lhsT=w2t[:, pp * 4 + fb, :],
                        rhs=embT[:, fb, pp, :],
                        start=(k == 0), stop=(k == 39),
                    )
                    k += 1
            lt_sb = work.tile([10, BSH], F32, tag="ltsb")
            nc.scalar.activation(lt_sb[:], lt_ps[:],
                                 mybir.ActivationFunctionType.Identity,
                                 bias=b2t[:, 0:1])
            # transpose to natural [b, 10] layout and store
            lg = work.tile([128, 4, 10], F32, tag="lg")
            for j in range(4):
                tp = tp_psum.tile([128, 10], F32)
                nc.tensor.transpose(tp[:], lt_sb[:, j * 128:(j + 1) * 128], id10[:])
                nc.vector.tensor_copy(lg[:, j, :], tp[:])
            nc.scalar.dma_start(logits_d.rearrange("(j p) n -> p j n", p=128), lg[:])

    nc.compile()
    return nc


def _get_nc():
    if "nc" not in _CACHE:
        _CACHE["nc"] = _build()
    return _CACHE["nc"]


def _prepare_in_maps(agent_context, lane_contexts, ngh_lane_context, ngh_contexts,
                     lane_label, W1, b1, W2, b2):
    agent_context = np.asarray(agent_context, np.float32)
    lane_contexts = np.asarray(lane_contexts, np.float32)
    ngh_lane_context = np.asarray(ngh_lane_context, np.float32)
    ngh_contexts = np.asarray(ngh_contexts, np.float32)
    lane_label = np.asarray(lane_label, np.int32)
    W1 = np.asarray(W1, np.float32)
    b1 = np.asarray(b1, np.float32)
    W2 = np.asarray(W2, np.float32)
    b2 = np.asarray(b2, np.float32)

    # replicated weights, pre-arranged for SBUF block layout
    w1_r = np.ascontiguousarray(
        W1.reshape(16, 128, 512).transpose(1, 0, 2)).astype(BF16)
    w2_r = np.ascontiguousarray(
        W2.reshape(40, 128, 10).transpose(1, 0, 2)).astype(BF16)
    b1_r = np.ascontiguousarray(b1.reshape(4, 128).T)
    b2_r = np.ascontiguousarray(b2.reshape(10, 1))

    in_maps = []
    for c in range(N_CORES):
        sl = slice(c * BSH, (c + 1) * BSH)
        lane = np.ascontiguousarray(lane_contexts[sl].reshape(RP, LD))
        nghl = np.ascontiguousarray(ngh_lane_context[sl].reshape(RP, LD))
        ngh = np.ascontiguousarray(ngh_contexts[sl].reshape(RP, LD))
        in_maps.append({
            "agent_n": np.ascontiguousarray(
                agent_context[sl].reshape(4, 128, LD).transpose(1, 0, 2)).astype(BF16),
            "lane_bf": lane.astype(BF16),
            "nghl_bf": nghl.astype(BF16),
            "ngh_bf": ngh.astype(BF16),
            "lane_f": lane,
            "nghl_f": nghl,
            "ngh_f": ngh,
            "label": np.ascontiguousarray(lane_label[sl]),
            "w1": w1_r, "w2": w2_r, "b1": b1_r, "b2": b2_r,
        })
    return in_maps


def _assemble(outs):

    logits = np.concatenate([outs[c]["logits"] for c in range(N_CORES)], axis=0)
    best_lane = np.concatenate([outs[c]["best_lane"] for c in range(N_CORES)], axis=0)
    best_nghl = np.concatenate([outs[c]["best_nghl"] for c in range(N_CORES)], axis=0)
    best_ngh = np.concatenate([outs[c]["best_ngh"] for c in range(N_CORES)], axis=0)
    return logits, best_lane, best_nghl, best_ngh


def _make_runner(nc):
    """Build a persistently-cached jitted SPMD runner (same mechanism as
    bass2jax.run_bass_via_pjrt, but the jit closure survives across calls so
    repeat invocations skip retracing)."""
    import jax
    from jax.sharding import Mesh, PartitionSpec
    from jax.experimental.shard_map import shard_map
    from concourse import bass2jax, mybir as mb

    bass2jax.install_neuronx_cc_hook()
    partition_name = nc.partition_id_tensor.name if nc.partition_id_tensor else None

    in_names, out_names, out_avals, zero_shapes = [], [], [], []
    for alloc in nc.m.functions[0].allocations:
        if not isinstance(alloc, mb.MemoryLocationSet):
            continue
        name = alloc.memorylocations[0].name
        if alloc.kind == "ExternalInput":
            if name != partition_name:
                in_names.append(name)
        elif alloc.kind == "ExternalOutput":
            shape = tuple(alloc.tensor_shape)
            dtype = mb.dt.np(alloc.dtype)
            out_names.append(name)
            out_avals.append(jax.core.ShapedArray(shape, dtype))
            zero_shapes.append((shape, dtype))
    n_params = len(in_names)
    n_outs = len(out_names)
    all_in_names = list(in_names) + list(out_names)
    if partition_name is not None:
        all_in_names.append(partition_name)

    def _body(*args):
        operands = list(args)
        if partition_name is not None:
            operands.append(bass2jax.partition_id_tensor())
        outs = bass2jax._bass_exec_p.bind(
            *operands,
            out_avals=tuple(out_avals),
            in_names=tuple(all_in_names),
            out_names=tuple(out_names),
            lowering_input_output_aliases=(),
            sim_require_finite=True,
            sim_require_nnan=True,
            nc=nc,
        )
        return tuple(outs)

    devices = jax.devices()[:N_CORES]
    mesh = Mesh(np.asarray(devices), ("core",))
    in_specs = (PartitionSpec("core"),) * (n_params + n_outs)
    out_specs = (PartitionSpec("core"),) * n_outs
    donate = tuple(range(n_params, n_params + n_outs))
    sharded = jax.jit(
        shard_map(_body, mesh=mesh, in_specs=in_specs, out_specs=out_specs,
                  check_rep=False),
        donate_argnums=donate, keep_unused=True)

    def run(in_maps):
        concat_in = [
            np.concatenate([np.asarray(in_maps[c][n]) for c in range(N_CORES)], axis=0)
            for n in in_names
        ]
        concat_zeros = [
            np.zeros((N_CORES * s[0], *s[1:]), d) for (s, d) in zero_shapes
        ]
        out_arrs = sharded(*concat_in, *concat_zeros)
        return [
            {n: np.asarray(out_arrs[i]).reshape(N_CORES, *zero_shapes[i][0])[c]
             for i, n in enumerate(out_names)}
            for c in range(N_CORES)
        ]

    return run


def _get_runner():
    if "runner" not in _CACHE:
        nc = _get_nc()
        try:
            _CACHE["runner"] = _make_runner(nc)
        except Exception:
            _CACHE["runner"] = None
    return _CACHE["runner"]


def kernel(**inputs):
    nc = _get_nc()
    in_maps = _prepare_in_maps(**inputs)
    runner = _get_runner()
    if runner is not None:
        try:
            return _assemble(runner(in_maps))
        except Exception:
            _CACHE["runner"] = None
    res = run_bass_kernel_spmd(nc, in_maps, core_ids=list(range(N_CORES)))
    return _assemble(res.results)


# revision 28
# speedup vs baseline: 1.1525x; 1.0131x over previous
"""Trainium2 Bass kernel for nn_ModeSelection (mode-selection MLP + one-hot gather).

Math (per batch row b, paths p=0..9):
  ctx[b,p,:]  = concat(agent[b], lane[b,p], ngh_lane[b,p], ngh[b,p])   # [2048]
  emb[b,p,:]  = relu(ctx[b,p,:] @ W1 + b1)                             # [512]
  logits[b,:] = emb[b].reshape(5120) @ W2 + b2                         # [10]
  idx[b]      = argmax(lane_label[b])  (first max)
  best_*[b]   = {lane, ngh_lane, ngh}[b, idx[b]]

Sharding: pure data parallel over batch (4096 -> 8 cores x 512 rows).
Weights replicated. No collectives.

Device strategy per core:
  - ctx parts are DMA'd straight from HBM in TRANSPOSED layout (bf16 XBAR
    DMA transpose) -> rhs operands [feat, rowpath] for the tensor engine.
  - mm1 accumulates over 16 K-blocks of 128 into PSUM [128, 320] tiles,
    relu+b1 applied by the scalar engine on the way to SBUF (bf16 embT).
  - mm2 contracts embT [feat, rowpath] with W2 into logitsT [10, 512],
    +b2 and a tiny PE transpose produce natural-layout logits.
  - best_* gathers: argmax computed on DVE from the labels, rows fetched
    by GPSIMD indirect DMA from the fp32 copies in HBM (bit-exact).
"""

import sys

sys.path.insert(0, "/opt/trn_rl_repo")

import numpy as np
import ml_dtypes

import concourse.bass as bass
import concourse.mybir as mybir
from concourse import bacc, tile
from concourse.bass_utils import run_bass_kernel_spmd
from concourse.masks import make_identity
from concourse.tile_rust import add_dep_helper

BF16 = ml_dtypes.bfloat16

N_CORES = 8
B = 4096          # full batch
BSH = B // N_CORES  # 512 batch rows per core
P_PATHS = 10
LD = 512          # feature dim of each ctx part
DIN = 2048
RP = BSH * P_PATHS  # 5120 rowpath (b,p) rows per core

NCHUNK = 8
CB = BSH // NCHUNK   # 64 batch rows per chunk
RC = CB * P_PATHS    # 640 rowpath rows per chunk
NG = 2               # matmul groups per chunk
GN = RC // NG        # 320 rowpath per matmul group

F32 = mybir.dt.float32
BF = mybir.dt.bfloat16
I32 = mybir.dt.int32

_CACHE = {}


def _build():
    nc = bacc.Bacc("TRN2", target_bir_lowering=False, debug=False,
                   enable_asserts=False, num_devices=N_CORES)

    # inputs
    agent_n = nc.dram_tensor("agent_n", [128, 4, LD], BF, kind="ExternalInput").ap()
    parts_bf = [
        nc.dram_tensor(name, [RP, LD], BF, kind="ExternalInput").ap()
        for name in ("lane_bf", "nghl_bf", "ngh_bf")
    ]
    gather_f = [
        nc.dram_tensor(name, [RP, LD], F32, kind="ExternalInput").ap()
        for name in ("lane_f", "nghl_f", "ngh_f")
    ]
    label = nc.dram_tensor("label", [BSH, P_PATHS], I32, kind="ExternalInput").ap()
    w1 = nc.dram_tensor("w1", [128, 16, 512], BF, kind="ExternalInput").ap()
    w2 = nc.dram_tensor("w2", [128, 40, 10], BF, kind="ExternalInput").ap()
    b1 = nc.dram_tensor("b1", [128, 4], F32, kind="ExternalInput").ap()
    b2 = nc.dram_tensor("b2", [10, 1], F32, kind="ExternalInput").ap()

    # outputs
    logits_d = nc.dram_tensor("logits", [BSH, P_PATHS], F32, kind="ExternalOutput").ap()
    best_d = [
        nc.dram_tensor(name, [BSH, LD], F32, kind="ExternalOutput").ap()
        for name in ("best_lane", "best_nghl", "best_ngh")
    ]

    with tile.TileContext(nc) as tc:
        with (
            tc.tile_pool(name="const", bufs=1) as const,
            tc.tile_pool(name="ctxt", bufs=3) as ctxt_pool,
            tc.tile_pool(name="mmps", bufs=6, space="PSUM") as mm_psum,
            tc.tile_pool(name="ltps", bufs=1, space="PSUM") as lt_psum,
            tc.tile_pool(name="tpps", bufs=1, space="PSUM") as tp_psum,
            tc.tile_pool(name="work", bufs=2) as work,
            tc.tile_pool(name="gat", bufs=6) as gat,
        ):
            # ---- startup. Plain loads only (W1 first: it gates all matmuls).
            # The agent matrix is transposed on the PE (not the XBAR) so the
            # chunk-0 DMA-transposes only have to wait for the plain loads to
            # drain once; everything else (W2, b2, labels, gather) is deferred
            # behind the last XBAR transpose — plain DMA traffic concurrent
            # with DMA-transposes serializes the XBAR pipeline.
            w1t = const.tile([128, 16, 512], BF)
            nc.scalar.dma_start(w1t[:], w1[:])
            ag_nat = const.tile([128, 4, LD], BF)
            nc.gpsimd.dma_start(ag_nat[:], agent_n[:])
            b1t = const.tile([128, 4], F32)
            nc.scalar.dma_start(b1t[:], b1[:])
            id10 = const.tile([10, 10], F32)
            make_identity(nc, id10[:])
            id128 = const.tile([128, 128], BF)
            make_identity(nc, id128[:])
            wvec = const.tile([128, 10], I32)  # [10, 9, ..., 1] per partition
            nc.gpsimd.iota(wvec[:], pattern=[[-1, 10]], base=10, channel_multiplier=0)
            embT = const.tile([128, 4, P_PATHS, BSH], BF)  # emb^T, path-major

            # ---- gather indices: labels land with the startup plain loads,
            # argmax chain runs on the then-idle DVE. Only the indirect
            # gathers + stores are deferred behind the XBAR transposes.
            rowidxs = []
            for blk in range(4):
                lblt = gat.tile([128, 10], I32, tag="lbl")
                nc.sync.dma_start(lblt[:], label[blk * 128:(blk + 1) * 128, :])
                score = gat.tile([128, 10], I32, tag="score")
                nc.vector.tensor_tensor(out=score[:], in0=lblt[:], in1=wvec[:],
                                        op=mybir.AluOpType.mult)
                m = gat.tile([128, 1], I32, tag="m")
                nc.vector.tensor_reduce(out=m[:], in_=score[:],
                                        axis=mybir.AxisListType.X,
                                        op=mybir.AluOpType.max)
                # idx = (10 - m) * min(m, 1): all-zero label row -> m=0 -> idx 0,
                # first set bit otherwise — matches argmax tie-breaking.
                u = gat.tile([128, 1], I32, tag="u")
                nc.vector.tensor_scalar(out=u[:], in0=m[:], scalar1=-1, scalar2=10,
                                        op0=mybir.AluOpType.mult,
                                        op1=mybir.AluOpType.add)
                gz = gat.tile([128, 1], I32, tag="gz")
                nc.vector.tensor_scalar(out=gz[:], in0=m[:], scalar1=1, scalar2=None,
                                        op0=mybir.AluOpType.min)
                idxv = gat.tile([128, 1], I32, tag="idx")
                nc.vector.tensor_tensor(out=idxv[:], in0=u[:], in1=gz[:],
                                        op=mybir.AluOpType.mult)
                base = gat.tile([128, 1], I32, tag="base")
                nc.gpsimd.iota(base[:], pattern=[[0, 1]], base=blk * 1280,
                               channel_multiplier=10)
                rowidx = gat.tile([128, 1], I32, tag="rowidx")
                nc.vector.tensor_tensor(out=rowidx[:], in0=idxv[:], in1=base[:],
                                        op=mybir.AluOpType.add)
                rowidxs.append(rowidx)

            # ---- agent transpose on PE + pre-matmul: agA[of, b] = agent @ W1[:512]
            agT = const.tile([128, 4, BSH], BF)
            for kb in range(4):
                tps = mm_psum.tile([128, BSH], BF, tag="ps")
                for s in range(4):
                    nc.tensor.transpose(
                        tps[:, s * 128:(s + 1) * 128],
                        ag_nat[:, s, kb * 128:(kb + 1) * 128], id128[:])
                if kb % 2 == 0:
                    nc.vector.tensor_copy(agT[:, kb, :], tps[:])
                else:
                    nc.scalar.copy(agT[:, kb, :], tps[:])
            agA = const.tile([128, 4, BSH], BF)
            for fbo in range(4):
                agps = mm_psum.tile([128, BSH], F32, tag="ps")
                for kb in range(4):
                    nc.tensor.matmul(
                        agps[:],
                        lhsT=w1t[:, kb, fbo * 128:(fbo + 1) * 128],
                        rhs=agT[:, kb, :],
                        start=(kb == 0), stop=(kb == 3),
                    )
                if fbo % 2 == 0:
                    nc.vector.tensor_copy(agA[:, fbo, :], agps[:])
                else:
                    nc.scalar.copy(agA[:, fbo, :], agps[:])

            # ---- mm1: embT[of, (b,p)] = relu(sum_part ctxT @ W1 + agA + b1) ----
            last_tp = None
            for c in range(NCHUNK):
                ctxTs = []
                for pi, part in enumerate(parts_bf):
                    t = ctxt_pool.tile([128, 4, RC], BF, tag=f"ctxt{pi}")
                    last_tp = nc.sync.dma_start_transpose(
                        t[:], part[c * RC:(c + 1) * RC, :])
                    ctxTs.append(t)
                # replicate this chunk's agent rows 10x (contiguous rhs for the
                # accumulating agent matmul below); step-0 reads are cheap on
                # DVE, fatal to matmul streaming rate
                agrep = ctxt_pool.tile([128, 4, RC], BF, tag="agrep")
                for fb in range(4):
                    ag_src = (agA[:, fb, c * CB:(c + 1) * CB]
                              .rearrange("p (b o) -> p b o", o=1)
                              .to_broadcast([128, CB, P_PATHS]))
                    nc.vector.tensor_copy(
                        agrep[:, fb, :].rearrange("p (b o) -> p b o", o=P_PATHS),
                        ag_src)
                for fbo in range(4):
                    for g in range(NG):
                        ps = mm_psum.tile([128, GN], F32)
                        k = 0
                        for pi, t in enumerate(ctxTs):
                            for kb in range(4):
                                nc.tensor.matmul(
                                    ps[:],
                                    lhsT=w1t[:, 4 + pi * 4 + kb,
                                             fbo * 128:(fbo + 1) * 128],
                                    rhs=t[:, kb, g * GN:(g + 1) * GN],
                                    start=(k == 0), stop=False,
                                )
                                k += 1
                        b0 = c * CB + g * (GN // P_PATHS)
                        # agent contribution via identity-stationary matmul
                        nc.tensor.matmul(
                            ps[:], lhsT=id128[:],
                            rhs=agrep[:, fbo, g * GN:(g + 1) * GN],
                            start=False, stop=True)
                        stage = ctxt_pool.tile([128, GN], BF, tag="stage")
                        nc.scalar.activation(
                            stage[:], ps[:],
                            mybir.ActivationFunctionType.Relu,
                            bias=b1t[:, fbo:fbo + 1],
                        )
                        nc.vector.tensor_copy(
                            embT[:, fbo, :, b0:b0 + GN // P_PATHS]
                            .rearrange("p t b -> p b t"), stage[:])

            def after_tp(instr, why):
                add_dep_helper(instr.ins, last_tp.ins, sync=True, reason=why)
                return instr

            # ---- deferred plain loads (post-transpose window) ----
            w2t = const.tile([128, 40, 10], BF)
            after_tp(nc.scalar.dma_start(w2t[:], w2[:]), "defer w2 after xbar")
            b2t = const.tile([10, 1], F32)
            after_tp(nc.scalar.dma_start(b2t[:], b2[:]), "defer b2 after xbar")

            # ---- deferred indirect gathers + stores (post-transpose) ----
            for blk in range(4):
                for ti, (src_t, dst) in enumerate(zip(gather_f, best_d)):
                    gt = gat.tile([128, LD], F32, tag="g")
                    after_tp(nc.gpsimd.indirect_dma_start(
                        out=gt[:], out_offset=None, in_=src_t[:],
                        in_offset=bass.IndirectOffsetOnAxis(
                            ap=rowidxs[blk][:, :1], axis=0)),
                        "defer gather after xbar")
                    nc.gpsimd.dma_start(dst[blk * 128:(blk + 1) * 128, :], gt[:])

            # ---- mm2: logitsT[10, b] = sum_{fb, pp} W2_blk^T @ embT_strided ----
            lt_ps = lt_psum.tile([10, BSH], F32, tag="ltmix")
            k = 0
            for fb in range(4):
                for pp in range(P_PATHS):
                    nc.tensor.matmul(
                        lt_ps[:],
                        lhsT=w2t[:, pp * 4 + fb, :],
                        rhs=embT[:, fb, pp, :],
                        start=(k == 0), stop=(k == 39),
                    )
                    k += 1
            lt_sb = work.tile([10, BSH], F32, tag="ltsb")
            nc.scalar.activation(lt_sb[:], lt_ps[:],
                                 mybir.ActivationFunctionType.Identity,
                                 bias=b2t[:, 0:1])
            # transpose to natural [b, 10] layout and store
            lg = work.tile([128, 4, 10], F32, tag="lg")
            for j in range(4):
                tp = tp_psum.tile([128, 10], F32)
                nc.tensor.transpose(tp[:], lt_sb[:, j * 128:(j + 1) * 128], id10[:])
                nc.vector.tensor_copy(lg[:, j, :], tp[:])
            nc.scalar.dma_start(logits_d.rearrange("(j p) n -> p j n", p=128), lg[:])

    nc.compile()
    return nc


def _get_nc():
    if "nc" not in _CACHE:
        _CACHE["nc"] = _build()
    return _CACHE["nc"]


def _prepare_in_maps(agent_context, lane_contexts, ngh_lane_context, ngh_contexts,
                     lane_label, W1, b1, W2, b2):
    agent_context = np.asarray(agent_context, np.float32)
    lane_contexts = np.asarray(lane_contexts, np.float32)
    ngh_lane_context = np.asarray(ngh_lane_context, np.float32)
    ngh_contexts = np.asarray(ngh_contexts, np.float32)
    lane_label = np.asarray(lane_label, np.int32)
    W1 = np.asarray(W1, np.float32)
    b1 = np.asarray(b1, np.float32)
    W2 = np.asarray(W2, np.float32)
    b2 = np.asarray(b2, np.float32)

    # replicated weights, pre-arranged for SBUF block layout
    w1_r = np.ascontiguousarray(
        W1.reshape(16, 128, 512).transpose(1, 0, 2)).astype(BF16)
    w2_r = np.ascontiguousarray(
        W2.reshape(40, 128, 10).transpose(1, 0, 2)).astype(BF16)
    b1_r = np.ascontiguousarray(b1.reshape(4, 128).T)
    b2_r = np.ascontiguousarray(b2.reshape(10, 1))

    in_maps = []
    for c in range(N_CORES):
        sl = slice(c * BSH, (c + 1) * BSH)
        lane = np.ascontiguousarray(lane_contexts[sl].reshape(RP, LD))
        nghl = np.ascontiguousarray(ngh_lane_context[sl].reshape(RP, LD))
        ngh = np.ascontiguousarray(ngh_contexts[sl].reshape(RP, LD))
        in_maps.append({
            "agent_n": np.ascontiguousarray(
                agent_context[sl].reshape(4, 128, LD).transpose(1, 0, 2)).astype(BF16),
            "lane_bf": lane.astype(BF16),
            "nghl_bf": nghl.astype(BF16),
            "ngh_bf": ngh.astype(BF16),
            "lane_f": lane,
            "nghl_f": nghl,
            "ngh_f": ngh,
            "label": np.ascontiguousarray(lane_label[sl]),
            "w1": w1_r, "w2": w2_r, "b1": b1_r, "b2": b2_r,
        })
    return in_maps


def _assemble(outs):

    logits = np.concatenate([outs[c]["logits"] for c in range(N_CORES)], axis=0)
    best_lane = np.concatenate([outs[c]["best_lane"] for c in range(N_CORES)], axis=0)
    best_nghl = np.concatenate([outs[c]["best_nghl"] for c in range(N_CORES)], axis=0)
    best_ngh = np.concatenate([outs[c]["best_ngh"] for c in range(N_CORES)], axis=0)
    return logits, best_lane, best_nghl, best_ngh


def _make_runner(nc):
    """Build a persistently-cached jitted SPMD runner (same mechanism as
    bass2jax.run_bass_via_pjrt, but the jit closure survives across calls so
    repeat invocations skip retracing)."""
    import jax
    from jax.sharding import Mesh, PartitionSpec
    from jax.experimental.shard_map import shard_map
    from concourse import bass2jax, mybir as mb

    bass2jax.install_neuronx_cc_hook()
    partition_name = nc.partition_id_tensor.name if nc.partition_id_tensor else None

    in_names, out_names, out_avals, zero_shapes = [], [], [], []
    for alloc in nc.m.functions[0].allocations:
        if not isinstance(alloc, mb.MemoryLocationSet):
            continue
        name = alloc.memorylocations[0].name
        if alloc.kind == "ExternalInput":
            if name != partition_name:
                in_names.append(name)
        elif alloc.kind == "ExternalOutput":
            shape = tuple(alloc.tensor_shape)
            dtype = mb.dt.np(alloc.dtype)
            out_names.append(name)
            out_avals.append(jax.core.ShapedArray(shape, dtype))
            zero_shapes.append((shape, dtype))
    n_params = len(in_names)
    n_outs = len(out_names)
    all_in_names = list(in_names) + list(out_names)
    if partition_name is not None:
        all_in_names.append(partition_name)

    def _body(*args):
        operands = list(args)
        if partition_name is not None:
            operands.append(bass2jax.partition_id_tensor())
        outs = bass2jax._bass_exec_p.bind(
            *operands,
            out_avals=tuple(out_avals),
            in_names=tuple(all_in_names),
            out_names=tuple(out_names),
            lowering_input_output_aliases=(),
            sim_require_finite=True,
            sim_require_nnan=True,
            nc=nc,
        )
        return tuple(outs)

    devices = jax.devices()[:N_CORES]
    mesh = Mesh(np.asarray(devices), ("core",))
    in_specs = (PartitionSpec("core"),) * (n_params + n_outs)
    out_specs = (PartitionSpec("core"),) * n_outs
    donate = tuple(range(n_params, n_params + n_outs))
    sharded = jax.jit(
        shard_map(_body, mesh=mesh, in_specs=in_specs, out_specs=out_specs,
                  check_rep=False),
        donate_argnums=donate, keep_unused=True)

    def run(in_maps):
        concat_in = [
            np.concatenate([np.asarray(in_maps[c][n]) for c in range(N_CORES)], axis=0)
            for n in in_names
        ]
        concat_zeros = [
            np.zeros((N_CORES * s[0], *s[1:]), d) for (s, d) in zero_shapes
        ]
        out_arrs = sharded(*concat_in, *concat_zeros)
        return [
            {n: np.asarray(out_arrs[i]).reshape(N_CORES, *zero_shapes[i][0])[c]
             for i, n in enumerate(out_names)}
            for c in range(N_CORES)
        ]

    return run


def _get_runner():
    if "runner" not in _CACHE:
        nc = _get_nc()
        try:
            _CACHE["runner"] = _make_runner(nc)
        except Exception:
            _CACHE["runner"] = None
    return _CACHE["runner"]


def kernel(**inputs):
    nc = _get_nc()
    in_maps = _prepare_in_maps(**inputs)
    runner = _get_runner()
    if runner is not None:
        try:
            return _assemble(runner(in_maps))
        except Exception:
            _CACHE["runner"] = None
    res = run_bass_kernel_spmd(nc, in_maps, core_ids=list(range(N_CORES)))
    return _assemble(res.results)
